# revision 1
# baseline (speedup 1.0000x reference)
"""Trn2 Bass kernel for nn_Attention_16793322128104.

Sharding: 8 cores = 2 batches x 4 head-groups (4 heads each).
Each core: QKV projection for its 768 Wqkv columns, 4 attention heads
(softmax with exact per-query max, folded into the S^T matmul as a 65th
contraction row), AV with ones-column denominator, partial out-projection.
Host sums the 4 head-group partials per batch and adds bout.
"""

import sys
from contextlib import ExitStack

import numpy as np

sys.path.insert(0, "/opt/trn_rl_repo")

import concourse.bass as bass
import concourse.bacc as bacc
import concourse.mybir as mybir
from concourse import tile
from concourse.bass_utils import run_bass_kernel_spmd

F32 = mybir.dt.float32
F32R = mybir.dt.float32r
F16 = mybir.dt.float16
BF16 = mybir.dt.bfloat16
IDENT = mybir.ActivationFunctionType.Identity

N_TOK = 2048          # tokens per batch
DIM = 1024            # model dim
NH = 4                # heads per core
DH = 64               # head dim
SCALE = 8.0           # sqrt(DH); reference MULTIPLIES by sqrt(d_head)

_CACHE = {}


def r32(ap):
    return ap.bitcast(F32R)


def build_nc():
    nc = bacc.Bacc()
    xt_d = nc.declare_dram_parameter("xt", [DIM + 1, N_TOK], F32R, isOutput=False)
    wg_d = nc.declare_dram_parameter("wg", [DIM + 1, 3 * NH * DH], F32R, isOutput=False)
    wout_d = nc.declare_dram_parameter("wout", [NH * DH, DIM], F16, isOutput=False)
    qkb_d = nc.declare_dram_parameter("qkb", [128, 4], F32, isOutput=False)
    id_d = nc.declare_dram_parameter("ident", [128, 128], F32, isOutput=False)
    out_d = nc.declare_dram_parameter("out", [DIM, N_TOK], F32, isOutput=True)

    with ExitStack() as ctx:
        tc = ctx.enter_context(tile.TileContext(nc))
        # ---------------- persistent pools ----------------
        qk_pool = ctx.enter_context(tc.tile_pool(name="qk", bufs=1))
        v_pool = ctx.enter_context(tc.tile_pool(name="v", bufs=1))
        misc_pool = ctx.enter_context(tc.tile_pool(name="misc", bufs=1))
        o2_pool = ctx.enter_context(tc.tile_pool(name="o2", bufs=1))
        psum = ctx.enter_context(
            tc.tile_pool(name="psum", bufs=2, space=bass.MemorySpace.PSUM)
        )

        # q2/k2: per-head [65, 2048]: rows 0:64 features, row 64 = shift/ones
        q2 = [qk_pool.tile([DH + 1, N_TOK], F32R, tag=f"q2{h}", name=f"q2{h}") for h in range(NH)]
        k2 = [qk_pool.tile([DH + 1, N_TOK], F32R, tag=f"k2{h}", name=f"k2{h}") for h in range(NH)]
        # v: per key-tile [128, NH, 65] fp16 (col 64 = ones -> denominator)
        vsb = [v_pool.tile([128, NH, DH + 1], F16, tag=f"v{m}", name=f"v{m}") for m in range(16)]
        ident = misc_pool.tile([128, 128], F32, tag="ident", name="identsb")
        ones1 = misc_pool.tile([1, DH], F32R, tag="ones1", name="ones1")
        qkb = misc_pool.tile([128, 4], F32, tag="qkb", name="qkbsb")
        negmax = [misc_pool.tile([16, 128], F32R, tag=f"nm{h}", name=f"nm{h}") for h in range(NH)]
        o2 = [o2_pool.tile([128, N_TOK], F16, tag=f"o2{t}", name=f"o2t{t}") for t in range(2)]

        nc.sync.dma_start(ident[:], id_d[:])
        nc.sync.dma_start(qkb[:], qkb_d[:])
        nc.sync.dma_start(ones1[:], xt_d[DIM : DIM + 1, 0:DH])
        for h in range(NH):
            nc.sync.dma_start(k2[h][DH : DH + 1, :], xt_d[DIM : DIM + 1, :])
        for m in range(16):
            nc.vector.memset(vsb[m][:, :, DH : DH + 1], 1.0)

        # ---------------- phase A: QKV projection ----------------
        with (
            tc.tile_pool(name="xt", bufs=1) as xt_pool,
            tc.tile_pool(name="wgp", bufs=1) as wg_pool,
        ):
            xt_all = xt_pool.tile([128, 8, N_TOK], F32R, tag="xta", name="xta")
            xt_row = xt_pool.tile([1, N_TOK], F32R, tag="xt8", name="xt8")
            wg_all = wg_pool.tile([128, 8, 768], F32R, tag="wga", name="wga")
            wg_row = wg_pool.tile([1, 768], F32R, tag="wg8", name="wg8")
            # chunked loads so compute starts after the first chunk lands
            for c in range(8):
                nc.sync.dma_start(wg_all[:, c, :], wg_d[c * 128 : (c + 1) * 128, :])
                nc.sync.dma_start(xt_all[:, c, :], xt_d[c * 128 : (c + 1) * 128, :])
            nc.sync.dma_start(xt_row[:], xt_d[DIM : DIM + 1, :])
            nc.sync.dma_start(wg_row[:], wg_d[DIM : DIM + 1, :])
            xt_sb = [xt_all[:, c, :] for c in range(8)] + [xt_row[:]]
            wg_sb = [wg_all[:, c, :] for c in range(8)] + [wg_row[:]]

            # q,k feature-major, chunk-outer waves: 8 accumulation regions
            # ([128,512] quarters of two mm-tag psum tiles) consume input
            # chunks as they arrive; bias folded into the psum->SBUF copy
            for wave in range(2):
                pw = [
                    psum.tile([128, N_TOK], F32, tag="mm", name="pw")
                    for _ in range(2)
                ]
                for c in range(8):
                    for r in range(8):
                        ft = 2 * wave + r // 4
                        tj = r % 4
                        col0 = ft * 128 if ft < 2 else 256 + (ft - 2) * 128
                        nc.tensor.matmul(
                            pw[r // 4][:, tj * 512 : (tj + 1) * 512],
                            wg_sb[c][:, col0 : col0 + 128],
                            xt_sb[c][:, tj * 512 : (tj + 1) * 512],
                            start=(c == 0),
                            stop=(c == 7),
                        )
                dst = q2 if wave == 0 else k2
                for r in range(8):
                    ft = 2 * wave + r // 4
                    tj = r % 4
                    hb = 2 * (ft % 2)
                    ts = slice(tj * 512, (tj + 1) * 512)
                    ps = pw[r // 4][:, ts]
                    nc.scalar.activation(
                        dst[hb][0:DH, ts], ps[0:DH], IDENT,
                        bias=qkb[0:DH, ft : ft + 1],
                    )
                    nc.scalar.activation(
                        dst[hb + 1][0:DH, ts], ps[DH:128], IDENT,
                        bias=qkb[DH:128, ft : ft + 1],
                    )

            # v token-major, chunk-outer: bank-aligned [*,512] regions (first
            # 256 cols used) — psum start/stop is bank-granular, so regions
            # of concurrently-accumulating groups must not share a bank
            for vw in range(2):
                pV = [
                    psum.tile([128, N_TOK], F32, tag="mm", name="pV")
                    for _ in range(2)
                ]
                for c in range(9):
                    for r in range(8):
                        tt = vw * 8 + r
                        rs = (r % 4) * 512
                        nc.tensor.matmul(
                            pV[r // 4][:, rs : rs + 256],
                            xt_sb[c][:, tt * 128 : (tt + 1) * 128],
                            wg_sb[c][:, 512:768],
                            start=(c == 0),
                            stop=(c == 8),
                        )
                for r in range(8):
                    tt = vw * 8 + r
                    rs = (r % 4) * 512
                    nc.scalar.copy(
                        vsb[tt][:, :, 0:DH],
                        pV[r // 4][:, rs : rs + 256].rearrange(
                            "p (h d) -> p h d", h=NH
                        ),
                    )

        # ---------------- phase B: attention per head ----------------
        with tc.tile_pool(name="pt", bufs=1) as pt_pool, tc.tile_pool(
            name="rp", bufs=1
        ) as r_pool, tc.tile_pool(name="mc", bufs=2) as mc_pool, tc.tile_pool(
            name="qbp", bufs=2
        ) as qb_pool:
            PT = pt_pool.tile([128, 16, N_TOK], F16, tag="PT", name="PT")
            qbs, kbs = {}, {}

            def mk_qbkb(h):
                """lazy bf16 copies of q/k features for head h's max pass
                (bf16 max err +-0.2 fits the fp16 PT +-1.37 window)."""
                qbs[h] = qb_pool.tile([DH, N_TOK], BF16, tag="qb", name=f"qb{h}")
                kbs[h] = qb_pool.tile([DH, N_TOK], BF16, tag="kb", name=f"kb{h}")
                nc.vector.tensor_copy(qbs[h][:], q2[h][0:DH, :].bitcast(F32))
                nc.vector.tensor_copy(kbs[h][:], k2[h][0:DH, :].bitcast(F32))

            mk_qbkb(0)
            for h in range(NH):
                # pass 1: S in [q, k] orientation (bf16) -> exact row max
                mc = mc_pool.tile([128, 16], F32, tag="mc", name="mc")
                for qt in range(16):
                    ps = psum.tile([128, N_TOK], F32, tag="mm", name="ps")
                    for kc in range(4):
                        nc.tensor.matmul(
                            ps[:, kc * 512 : (kc + 1) * 512],
                            qbs[h][:, qt * 128 : (qt + 1) * 128],
                            kbs[h][:, kc * 512 : (kc + 1) * 512],
                            start=True,
                            stop=True,
                        )
                    nc.vector.reduce_max(
                        mc[:, qt : qt + 1], ps[:], axis=mybir.AxisListType.X
                    )
                if h + 1 < NH:
                    mk_qbkb(h + 1)
                # transpose maxes to a row, negate, DMA into q2 row 64
                pst = psum.tile([16, 128], F32, tag="mm", name="pst")
                nc.tensor.transpose(pst[:], mc[:], ident[:])
                nc.vector.tensor_scalar_mul(negmax[h][:], pst[:], -1.0)
                nc.sync.dma_start(q2[h][DH : DH + 1, :], negmax[h][:])

                # pass 2: S^T with shift folded in; exp -> fp16 P^T
                for m in range(16):
                    ps = psum.tile([128, N_TOK], F32, tag="mm", name="ps")
                    for j in range(4):
                        nc.tensor.matmul(
                            ps[:, j * 512 : (j + 1) * 512],
                            k2[h][:, m * 128 : (m + 1) * 128],
                            q2[h][:, j * 512 : (j + 1) * 512],
                            start=True,
                            stop=True,
                        )
                    nc.scalar.activation(
                        PT[:, m, :], ps[:], mybir.ActivationFunctionType.Exp,
                        scale=SCALE,
                    )

                # AV: o^T[d, t] + denominator row
                po = psum.tile([DH + 1, N_TOK], F32, tag="mm", name="po")
                for j in range(4):
                    for m in range(16):
                        nc.tensor.matmul(
                            po[:, j * 512 : (j + 1) * 512],
                            vsb[m][:, h, :],
                            PT[:, m, j * 512 : (j + 1) * 512],
                            start=(m == 0),
                            stop=(m == 15),
                        )
                # normalize: o2 rows = o^T * (1/denom) broadcast via K=1 matmul
                rr0 = r_pool.tile([1, N_TOK], F32, tag="rr0", name="rr0")
                rr = r_pool.tile([1, N_TOK], F32R, tag="rr", name="rr")
                rm = r_pool.tile([DH, N_TOK], F32, tag="rm", name="rm")
                nc.vector.reciprocal(rr0[:], po[DH : DH + 1, :])
                nc.vector.tensor_copy(rr[:], rr0[:])
                pr = psum.tile([DH, N_TOK], F32, tag="mm", name="pr")
                for j in range(4):
                    nc.tensor.matmul(
                        pr[:, j * 512 : (j + 1) * 512],
                        ones1[:],
                        rr[:, j * 512 : (j + 1) * 512],
                        start=True,
                        stop=True,
                    )
                nc.vector.tensor_copy(rm[:], pr[:])
                o2dst = o2[h // 2][DH * (h % 2) : DH * (h % 2) + DH, :]
                nc.vector.tensor_mul(o2dst, po[0:DH, :], rm[:])

        # ---------------- phase C: out projection ----------------
        with tc.tile_pool(name="ob", bufs=3) as ob_pool, tc.tile_pool(
            name="wop", bufs=1
        ) as wo_pool:
            wout_sb = [wo_pool.tile([128, DIM], F16, tag=f"wo{t}", name=f"wo{t}") for t in range(2)]
            for t in range(2):
                nc.sync.dma_start(wout_sb[t][:], wout_d[t * 128 : (t + 1) * 128, :])
            for dc in range(8):
                for j in range(4):
                    ps = psum.tile([128, 512], F32, tag="mm", name="ps")
                    for ht in range(2):
                        nc.tensor.matmul(
                            ps[:],
                            wout_sb[ht][:, dc * 128 : (dc + 1) * 128],
                            o2[ht][:, j * 512 : (j + 1) * 512],
                            start=(ht == 0),
                            stop=(ht == 1),
                        )
                    ob = ob_pool.tile([128, 512], F32, tag="ob", name="ob")
                    nc.vector.tensor_copy(ob[:], ps[:])
                    nc.sync.dma_start(
                        out_d[dc * 128 : (dc + 1) * 128, j * 512 : (j + 1) * 512],
                        ob[:],
                    )
    nc.finalize()
    return nc


def _get_nc():
    if "nc" not in _CACHE:
        _CACHE["nc"] = build_nc()
    return _CACHE["nc"]


def kernel(x, Wqkv, bqkv, Wout, bout):
    x = np.asarray(x, np.float32)
    Wqkv = np.asarray(Wqkv, np.float32)
    bqkv = np.asarray(bqkv, np.float32)
    Wout = np.asarray(Wout, np.float32)
    bout = np.asarray(bout, np.float32)
    B = x.shape[0]
    ident = np.eye(128, dtype=np.float32)
    ones_row = np.ones((1, N_TOK), np.float32)

    in_maps = []
    for c in range(8):
        b, g = c // 4, c % 4
        xt = np.concatenate([np.ascontiguousarray(x[b].T), ones_row], 0)
        cols = []
        bias = []
        for blk in range(3):  # q, k, v column blocks of Wqkv
            s = blk * DIM + g * NH * DH
            cols.append(Wqkv[:, s : s + NH * DH])
            bias.append(bqkv[s : s + NH * DH])
        wg = np.concatenate(
            [np.concatenate(cols, 1), np.concatenate(bias)[None, :]], 0
        )
        # per-partition bias for the q/k activation copies: [128, 4] with
        # column ft = bias for that 128-feature block (ft 0,1 = q; 2,3 = k)
        qkb = np.stack(
            [
                bqkv[g * 256 : g * 256 + 128],
                bqkv[g * 256 + 128 : g * 256 + 256],
                bqkv[DIM + g * 256 : DIM + g * 256 + 128],
                bqkv[DIM + g * 256 + 128 : DIM + g * 256 + 256],
            ],
            axis=1,
        ).astype(np.float32)
        wo = np.ascontiguousarray(
            Wout[g * NH * DH : (g + 1) * NH * DH, :]
        ).astype(np.float16)
        in_maps.append(
            {
                "xt": np.ascontiguousarray(xt),
                "wg": np.ascontiguousarray(wg),
                "wout": wo,
                "qkb": np.ascontiguousarray(qkb),
                "ident": ident,
            }
        )

    _CACHE["last_in_maps"] = in_maps
    res = run_bass_kernel_spmd(_get_nc(), in_maps, list(range(8))).results
    out = np.empty((B, N_TOK, DIM), np.float32)
    for b in range(B):
        acc = res[4 * b]["out"].astype(np.float32)
        for g in range(1, 4):
            acc = acc + res[4 * b + g]["out"]
        out[b] = acc.T + bout[None, :]
    return out


if __name__ == "__main__":
    rng = np.random.default_rng(0)
    x = rng.standard_normal((2, N_TOK, DIM), np.float32)
    Wqkv = rng.standard_normal((DIM, 3 * DIM), np.float32) * DIM**-0.5
    bqkv = rng.standard_normal((3 * DIM,), np.float32) * 0.02
    Wout = rng.standard_normal((DIM, DIM), np.float32) * DIM**-0.5
    bout = rng.standard_normal((DIM,), np.float32) * 0.02
    o = kernel(x=x, Wqkv=Wqkv, bqkv=bqkv, Wout=Wout, bout=bout)
    print("kernel ran, out shape", o.shape)



# revision 12
# speedup vs baseline: 1.2366x; 1.2366x over previous
"""Trn2 Bass kernel for nn_Attention_16793322128104.

Sharding: 8 cores = 2 batches x 4 head-groups (4 heads each).
Per core: fp16 QKV projection (768 Wqkv cols), 4 attention heads with
exact per-query max (pass-1 fp32r S + vector reduce_max), softmax shift
folded into the S^T matmul as a 65th contraction row, exp on the scalar
engine into an fp16 PT ring, AV with ones-column denominator, fp16
partial out-projection. Host sums the 4 head-group partials per batch.

Schedule: uniform [128,1024] PSUM stream tiles (3-slot rotation, 6
banks) + a 2-bank AV accumulator pool. Pass-1 max for head h+2 and
AV for head h-1 are software-pipelined into head h's S^T/exp phase so
tensor/vector/scalar engines all stay busy.
"""

import sys
from contextlib import ExitStack

import numpy as np

sys.path.insert(0, "/opt/trn_rl_repo")

import concourse.bass as bass
import concourse.bacc as bacc
import concourse.mybir as mybir
from concourse import tile
from concourse.bass_utils import run_bass_kernel_spmd

F32 = mybir.dt.float32
F32R = mybir.dt.float32r
F16 = mybir.dt.float16
IDENT = mybir.ActivationFunctionType.Identity
EXP = mybir.ActivationFunctionType.Exp
XAX = mybir.AxisListType.X

N_TOK = 2048
DIM = 1024
NH = 4                # heads per core
DH = 64               # head dim
SCALE = 8.0           # sqrt(DH); reference MULTIPLIES by sqrt(d_head)
RING = 24             # PT ring slots (16 per head, AV trails one head)

_CACHE = {}


def _rs(h, m):
    return (16 * h + m) % RING


def build_nc():
    nc = bacc.Bacc()
    xt_d = nc.declare_dram_parameter("xt", [DIM + 1, N_TOK], F16, isOutput=False)
    wg_d = nc.declare_dram_parameter("wg", [DIM + 1, 3 * NH * DH], F16, isOutput=False)
    qkb_d = nc.declare_dram_parameter("qkb", [128, 4], F32, isOutput=False)
    wo_d = nc.declare_dram_parameter("wout", [2 * 128, DIM], F16, isOutput=False)
    id_d = nc.declare_dram_parameter("ident", [128, 128], F32, isOutput=False)
    on32_d = nc.declare_dram_parameter("ones32", [1, N_TOK], F32R, isOutput=False)
    on16_d = nc.declare_dram_parameter("ones16", [1, DH], F16, isOutput=False)
    out_d = nc.declare_dram_parameter("out", [DIM, N_TOK], F16, isOutput=True)

    with ExitStack() as ctx:
        tc = ctx.enter_context(tile.TileContext(nc))
        pers = ctx.enter_context(tc.tile_pool(name="pers", bufs=1))
        ps = ctx.enter_context(
            tc.tile_pool(name="ps", bufs=3, space=bass.MemorySpace.PSUM)
        )
        pop = ctx.enter_context(
            tc.tile_pool(name="pop", bufs=1, space=bass.MemorySpace.PSUM)
        )

        q2 = [pers.tile([DH + 1, N_TOK], F32R, tag=f"q2{h}", name=f"q2{h}") for h in range(NH)]
        k2 = [pers.tile([DH + 1, N_TOK], F32R, tag=f"k2{h}", name=f"k2{h}") for h in range(NH)]
        vsb = [pers.tile([128, NH, DH + 1], F16, tag=f"v{m}", name=f"v{m}") for m in range(16)]
        o2 = [pers.tile([128, N_TOK], F16, tag=f"o2{t}", name=f"o2{t}") for t in range(2)]
        wo_sb = [pers.tile([128, DIM], F16, tag=f"wo{t}", name=f"wo{t}") for t in range(2)]
        ident = pers.tile([128, 128], F32, tag="id", name="identsb")
        qkb = pers.tile([128, 4], F32, tag="qkb", name="qkbsb")
        on16 = pers.tile([1, DH], F16, tag="on16", name="on16sb")
        mc2 = [pers.tile([128, 16, 2], F32, tag=f"mc{h}", name=f"mc{h}") for h in range(NH)]
        mcf = [pers.tile([128, 16], F32, tag=f"mcf{h}", name=f"mcf{h}") for h in range(NH)]
        negm = [pers.tile([16, 128], F32R, tag=f"nm{h}", name=f"nm{h}") for h in range(NH)]
        # norm temporaries (tag-reused per (h, j) with 2-way ping)
        dns = [pers.tile([1, N_TOK // 2], F16, tag=f"dns{p}", name=f"dns{p}") for p in range(2)]
        dnf = [pers.tile([128, 8], F16, tag=f"dnf{p}", name=f"dnf{p}") for p in range(2)]
        dnr = [pers.tile([128, 8], F32, tag=f"dnr{p}", name=f"dnr{p}") for p in range(2)]
        dnh = [pers.tile([128, 8], F16, tag=f"dnh{p}", name=f"dnh{p}") for p in range(2)]
        rrj = [pers.tile([1, N_TOK // 2], F16, tag=f"rr{p}", name=f"rr{p}") for p in range(2)]
        rm = [pers.tile([DH, N_TOK // 2], F16, tag=f"rm{p}", name=f"rm{p}") for p in range(2)]

        nc.sync.dma_start(ident[:], id_d[:])
        nc.sync.dma_start(qkb[:], qkb_d[:])
        nc.sync.dma_start(on16[:], on16_d[:])
        for h in range(NH):
            nc.sync.dma_start(k2[h][DH : DH + 1, :], on32_d[:])
        for m in range(16):
            nc.vector.memset(vsb[m][:, :, DH : DH + 1], 1.0)
        for t in range(2):
            nc.sync.dma_start(wo_sb[t][:], wo_d[t * 128 : (t + 1) * 128, :])

        # ---------------- pass-1 helper: per-query max for head h ---------
        def p1_qt(h, qt):
            for half in range(2):
                p = ps.tile([128, N_TOK // 2], F32, tag="s", name="p1")
                for kc in range(2):
                    nc.tensor.matmul(
                        p[:, kc * 512 : (kc + 1) * 512],
                        q2[h][0:DH, qt * 128 : (qt + 1) * 128],
                        k2[h][0:DH, half * 1024 + kc * 512 : half * 1024 + (kc + 1) * 512],
                        start=True,
                        stop=True,
                    )
                nc.vector.reduce_max(mc2[h][:, qt, half : half + 1], p[:], axis=XAX)
            if qt == 15:
                nc.vector.reduce_max(mcf[h][:], mc2[h][:], axis=XAX)
                pst = ps.tile([128, N_TOK // 2], F32, tag="s", name="pst")
                nc.tensor.transpose(pst[0:16, 0:128], mcf[h][:], ident[:])
                nc.vector.tensor_scalar_mul(negm[h][:], pst[0:16, 0:128], -1.0)
                nc.sync.dma_start(q2[h][DH : DH + 1, :], negm[h][:])

        # ---------------- AV + normalize helpers --------------------------
        def av_j_mm(h, j, m2, po):
            for u in range(2):  # fp16 moving operand caps at 512 cols
                nc.tensor.matmul(
                    po[:, u * 512 : (u + 1) * 512],
                    vsb[m2][:, h, :],
                    PT[:, _rs(h, m2), j * 1024 + u * 512 : j * 1024 + (u + 1) * 512],
                    start=(m2 == 0),
                    stop=(m2 == 15),
                )

        def norm_j(h, j, po):
            p = (2 * h + j) % 2
            nc.scalar.copy(dns[p][:], po[DH : DH + 1, :])
            nc.sync.dma_start(dnf[p][:], dns[p][:])
            nc.vector.reciprocal(dnr[p][:], dnf[p][:])
            nc.vector.tensor_copy(dnh[p][:], dnr[p][:])
            nc.sync.dma_start(rrj[p][:], dnh[p][:])
            pr = ps.tile([128, N_TOK // 2], F32, tag="s", name="pr")
            for u in range(2):
                nc.tensor.matmul(
                    pr[0:DH, u * 512 : (u + 1) * 512],
                    on16[:], rrj[p][:, u * 512 : (u + 1) * 512],
                    start=True, stop=True,
                )
            # tensor_tensor may read only one PSUM operand: bounce pr to SBUF
            nc.scalar.copy(rm[p][:], pr[0:DH, :])
            o2dst = o2[h // 2][(h % 2) * DH : (h % 2) * DH + DH, j * 1024 : (j + 1) * 1024]
            nc.vector.tensor_mul(o2dst, po[0:DH, :], rm[p][:])

        # ---------------- phase A: QKV projection (fp16) -------------------
        with (
            tc.tile_pool(name="xt", bufs=1) as xt_pool,
            tc.tile_pool(name="wgp", bufs=1) as wg_pool,
        ):
            xt_sb = xt_pool.tile([128, 8, N_TOK], F16, tag="xta", name="xta")
            xtr = xt_pool.tile([1, N_TOK], F16, tag="xtr", name="xtr")
            wg_sb = wg_pool.tile([128, 8, 3 * NH * DH], F16, tag="wga", name="wga")
            wgr = wg_pool.tile([1, 3 * NH * DH], F16, tag="wgr", name="wgr")
            for c in range(8):
                nc.sync.dma_start(wg_sb[:, c, :], wg_d[c * 128 : (c + 1) * 128, :])
                nc.sync.dma_start(xt_sb[:, c, :], xt_d[c * 128 : (c + 1) * 128, :])
            nc.sync.dma_start(xtr[:], xt_d[DIM : DIM + 1, :])
            nc.sync.dma_start(wgr[:], wg_d[DIM : DIM + 1, :])

            # q/k waves: ft 0/1 = q heads01/23 (cols 0:256), 2/3 = k (256:512)
            def qk_wave(ft):
                dst = q2 if ft < 2 else k2
                hb = 2 * (ft % 2)
                pw = [ps.tile([128, N_TOK // 2], F32, tag="s", name="pw") for _ in range(2)]
                for c in range(8):
                    for half in range(2):
                        for tj in range(2):
                            nc.tensor.matmul(
                                pw[half][:, tj * 512 : (tj + 1) * 512],
                                wg_sb[:, c, ft * 128 : (ft + 1) * 128],
                                xt_sb[:, c, half * 1024 + tj * 512 : half * 1024 + (tj + 1) * 512],
                                start=(c == 0),
                                stop=(c == 7),
                            )
                for half in range(2):
                    cols = slice(half * 1024, (half + 1) * 1024)
                    nc.scalar.activation(
                        dst[hb][0:DH, cols], pw[half][0:DH, :], IDENT,
                        bias=qkb[0:DH, ft : ft + 1],
                    )
                    nc.scalar.activation(
                        dst[hb + 1][0:DH, cols], pw[half][DH:128, :], IDENT,
                        bias=qkb[DH:128, ft : ft + 1],
                    )

            def v_tile(i):
                # two token-tiles (2i, 2i+1) of v into one psum tile
                pv = ps.tile([128, N_TOK // 2], F32, tag="s", name="pv")
                for c in range(9):
                    for u in range(2):
                        tt = 2 * i + u
                        lhs = (
                            xt_sb[:, c, tt * 128 : (tt + 1) * 128]
                            if c < 8
                            else xtr[:, tt * 128 : (tt + 1) * 128]
                        )
                        rhs = (
                            wg_sb[:, c, 2 * NH * DH : 3 * NH * DH]
                            if c < 8
                            else wgr[:, 2 * NH * DH : 3 * NH * DH]
                        )
                        nc.tensor.matmul(
                            pv[:, u * 512 : u * 512 + NH * DH],
                            lhs, rhs,
                            start=(c == 0),
                            stop=(c == 8),
                        )
                for u in range(2):
                    tt = 2 * i + u
                    nc.scalar.copy(
                        vsb[tt][:, :, 0:DH],
                        pv[:, u * 512 : u * 512 + NH * DH].rearrange(
                            "p (h d) -> p h d", h=NH
                        ),
                    )

            qk_wave(0)          # q heads 0,1
            qk_wave(2)          # k heads 0,1
            # interleave q23/k23 waves and v tiles with pass-1 of heads 0,1
            qk_wave(1)
            p1_qt(0, 0)
            qk_wave(3)
            p1_qt(0, 1)
            for i in range(8):
                v_tile(i)
                if i < 7:
                    p1_qt(0, 2 + 2 * i)
                    p1_qt(0, 3 + 2 * i)
            for qt in range(16):
                p1_qt(1, qt)

        # ---------------- attention: S^T + exp, pipelined ------------------
        # PT ring lives in its own pool so it reuses the SBUF freed by the
        # xt/wg pools above.
        ptp = ctx.enter_context(tc.tile_pool(name="ptp", bufs=1))
        PT = ptp.tile([128, RING, N_TOK], F16, tag="PT", name="PTsb")

        av_state = {}

        def pp(h):
            hp = h + 2 if h < 2 else None     # pass-1 head piggybacked
            ha = h - 1 if h >= 1 else None    # AV head piggybacked
            po = None
            for m in range(16):
                # AV/norm for h-1 first: exp(h, m) reuses the PT ring slot
                # that AV(h-1) reads at this step, so its readers must be
                # emitted ahead of the overwrite.
                if ha is not None:
                    j = m // 8
                    if m % 8 == 0:
                        if m == 8:
                            norm_j(ha, 0, po)
                        po = pop.tile([DH + 1, N_TOK // 2], F32, tag="po", name="po")
                        av_state[ha] = po
                    av_j_mm(ha, j, 2 * (m % 8), po)
                    av_j_mm(ha, j, 2 * (m % 8) + 1, po)
                for half in range(2):
                    st = ps.tile([128, N_TOK // 2], F32, tag="s", name="st")
                    for j2 in range(2):
                        nc.tensor.matmul(
                            st[:, j2 * 512 : (j2 + 1) * 512],
                            k2[h][:, m * 128 : (m + 1) * 128],
                            q2[h][:, half * 1024 + j2 * 512 : half * 1024 + (j2 + 1) * 512],
                            start=True,
                            stop=True,
                        )
                    nc.scalar.activation(
                        PT[:, _rs(h, m), half * 1024 : (half + 1) * 1024],
                        st[:], EXP, scale=SCALE,
                    )
                if hp is not None:
                    p1_qt(hp, m)
            if ha is not None:
                norm_j(ha, 1, po)

        for h in range(NH):
            pp(h)

        # tail: AV + norm for head 3
        for j in range(2):
            po = pop.tile([DH + 1, N_TOK // 2], F32, tag="po", name="po")
            for m2 in range(16):
                av_j_mm(3, j, m2, po)
            norm_j(3, j, po)

        # ---------------- out projection -----------------------------------
        with tc.tile_pool(name="ob", bufs=3) as obp:
            for dc in range(8):
                for half in range(2):
                    pout = ps.tile([128, N_TOK // 2], F32, tag="s", name="pout")
                    for ht in range(2):
                        for u in range(2):
                            nc.tensor.matmul(
                                pout[:, u * 512 : (u + 1) * 512],
                                wo_sb[ht][:, dc * 128 : (dc + 1) * 128],
                                o2[ht][:, half * 1024 + u * 512 : half * 1024 + (u + 1) * 512],
                                start=(ht == 0),
                                stop=(ht == 1),
                            )
                    ob = obp.tile([128, N_TOK // 2], F16, tag="ob", name="ob")
                    nc.scalar.copy(ob[:], pout[:])
                    nc.sync.dma_start(
                        out_d[dc * 128 : (dc + 1) * 128, half * 1024 : (half + 1) * 1024],
                        ob[:],
                    )
    nc.finalize()
    return nc


def _get_nc():
    if "nc" not in _CACHE:
        _CACHE["nc"] = build_nc()
    return _CACHE["nc"]


def kernel(x, Wqkv, bqkv, Wout, bout):
    x = np.asarray(x, np.float32)
    Wqkv = np.asarray(Wqkv, np.float32)
    bqkv = np.asarray(bqkv, np.float32)
    Wout = np.asarray(Wout, np.float32)
    bout = np.asarray(bout, np.float32)
    B = x.shape[0]
    ident = np.eye(128, dtype=np.float32)
    ones_row16 = np.ones((1, N_TOK), np.float16)

    in_maps = []
    for c in range(8):
        b, g = c // 4, c % 4
        xt = np.concatenate(
            [np.ascontiguousarray(x[b].T).astype(np.float16), ones_row16], 0
        )
        cols, bias = [], []
        for blk in range(3):  # q, k, v column blocks of Wqkv
            s = blk * DIM + g * NH * DH
            cols.append(Wqkv[:, s : s + NH * DH])
            bias.append(bqkv[s : s + NH * DH])
        wg = np.concatenate(
            [np.concatenate(cols, 1), np.concatenate(bias)[None, :]], 0
        ).astype(np.float16)
        # per-partition bias for q/k copies: col ft = 128-feature block
        # (ft 0,1 = q heads01/23; ft 2,3 = k heads01/23)
        qb = bqkv[g * 256 : (g + 1) * 256]
        kb = bqkv[DIM + g * 256 : DIM + (g + 1) * 256]
        qkb = np.stack([qb[:128], qb[128:], kb[:128], kb[128:]], 1).astype(np.float32)
        wo = np.ascontiguousarray(Wout[g * NH * DH : (g + 1) * NH * DH, :]).astype(
            np.float16
        )
        in_maps.append(
            {
                "xt": np.ascontiguousarray(xt),
                "wg": np.ascontiguousarray(wg),
                "qkb": np.ascontiguousarray(qkb),
                "wout": wo,
                "ident": ident,
                "ones32": np.ones((1, N_TOK), np.float32),
                "ones16": np.ones((1, DH), np.float16),
            }
        )

    _CACHE["last_in_maps"] = in_maps
    res = run_bass_kernel_spmd(_get_nc(), in_maps, list(range(8))).results
    out = np.empty((B, N_TOK, DIM), np.float32)
    for b in range(B):
        acc = res[4 * b]["out"].astype(np.float32)
        for g in range(1, 4):
            acc = acc + res[4 * b + g]["out"].astype(np.float32)
        out[b] = acc.T + bout[None, :]
    return out


if __name__ == "__main__":
    rng = np.random.default_rng(0)
    x = rng.standard_normal((2, N_TOK, DIM)).astype(np.float32)
    Wqkv = (rng.standard_normal((DIM, 3 * DIM)) * DIM**-0.5).astype(np.float32)
    bqkv = (rng.standard_normal(3 * DIM) * 0.02).astype(np.float32)
    Wout = (rng.standard_normal((DIM, DIM)) * DIM**-0.5).astype(np.float32)
    bout = (rng.standard_normal(DIM) * 0.02).astype(np.float32)
    o = kernel(x=x, Wqkv=Wqkv, bqkv=bqkv, Wout=Wout, bout=bout)
    print("kernel ran, out shape", o.shape)


# revision 19
# speedup vs baseline: 1.5086x; 1.2200x over previous
"""Trn2 Bass kernel for nn_Attention_16793322128104.

Sharding: 8 cores = 2 batches x 4 head-groups (4 heads each).
Per core: fp16 QKV projection (768 Wqkv cols), 4 attention heads with
exact per-query max (pass-1 fp32r S + vector reduce_max), softmax shift
folded into the S^T matmul as a 65th contraction row, exp on the scalar
engine into an fp16 PT ring, AV with ones-column denominator, fp16
partial out-projection. Host sums the 4 head-group partials per batch.

Schedule: uniform [128,1024] PSUM stream tiles (3-slot rotation, 6
banks) + a 2-bank AV accumulator pool. Pass-1 max for head h+2 and
AV for head h-1 are software-pipelined into head h's S^T/exp phase so
tensor/vector/scalar engines all stay busy.
"""

import sys
from contextlib import ExitStack

import numpy as np

sys.path.insert(0, "/opt/trn_rl_repo")

import concourse.bass as bass
import concourse.bacc as bacc
import concourse.mybir as mybir
from concourse import tile
from concourse.bass_utils import run_bass_kernel_spmd

F32 = mybir.dt.float32
F32R = mybir.dt.float32r
F16 = mybir.dt.float16
IDENT = mybir.ActivationFunctionType.Identity
EXP = mybir.ActivationFunctionType.Exp
XAX = mybir.AxisListType.X

N_TOK = 2048
DIM = 1024
NH = 4                # heads per core
DH = 64               # head dim
SCALE = 8.0           # sqrt(DH); reference MULTIPLIES by sqrt(d_head)
RING = 24             # PT ring slots (16 per head, AV trails one head)

_CACHE = {}


def _rs(h, m):
    return (16 * h + m) % RING


def build_nc():
    nc = bacc.Bacc()
    xt_d = nc.declare_dram_parameter("xt", [DIM + 1, N_TOK], F16, isOutput=False)
    wg_d = nc.declare_dram_parameter("wg", [DIM + 1, 3 * NH * DH], F16, isOutput=False)
    qkb_d = nc.declare_dram_parameter("qkb", [128, 4], F32, isOutput=False)
    wo_d = nc.declare_dram_parameter("wout", [2 * 128, DIM], F16, isOutput=False)
    id_d = nc.declare_dram_parameter("ident", [128, 128], F32, isOutput=False)
    on32_d = nc.declare_dram_parameter("ones32", [1, N_TOK], F32R, isOutput=False)
    on16_d = nc.declare_dram_parameter("ones16", [1, DH], F16, isOutput=False)
    out_d = nc.declare_dram_parameter("out", [DIM, N_TOK], F16, isOutput=True)

    with ExitStack() as ctx:
        tc = ctx.enter_context(tile.TileContext(nc))
        pers = ctx.enter_context(tc.tile_pool(name="pers", bufs=1))
        ps = ctx.enter_context(
            tc.tile_pool(name="ps", bufs=3, space=bass.MemorySpace.PSUM)
        )
        pop = ctx.enter_context(
            tc.tile_pool(name="pop", bufs=1, space=bass.MemorySpace.PSUM)
        )

        q2 = [pers.tile([DH + 1, N_TOK], F32R, tag=f"q2{h}", name=f"q2{h}") for h in range(NH)]
        k2 = [pers.tile([DH + 1, N_TOK], F32R, tag=f"k2{h}", name=f"k2{h}") for h in range(NH)]
        vsb = [pers.tile([128, NH, DH + 1], F16, tag=f"v{m}", name=f"v{m}") for m in range(16)]
        o2 = [pers.tile([128, N_TOK], F16, tag=f"o2{t}", name=f"o2{t}") for t in range(2)]
        wo_sb = [pers.tile([128, DIM], F16, tag=f"wo{t}", name=f"wo{t}") for t in range(2)]
        ident = pers.tile([128, 128], F32, tag="id", name="identsb")
        qkb = pers.tile([128, 4], F32, tag="qkb", name="qkbsb")
        on16 = pers.tile([1, DH], F16, tag="on16", name="on16sb")
        mc2 = [pers.tile([128, 16, 2], F32, tag=f"mc{h}", name=f"mc{h}") for h in range(NH)]
        mcf = [pers.tile([128, 16], F32, tag=f"mcf{h}", name=f"mcf{h}") for h in range(NH)]
        negm = [
            [pers.tile([8, 128], F32R, tag=f"nm{h}{u}", name=f"nm{h}{u}") for u in range(2)]
            for h in range(NH)
        ]
        # deferred-normalization temporaries
        denall = pers.tile([8, N_TOK // 2], F32, tag="denall", name="denall")
        dtmp = [pers.tile([1, N_TOK // 2], F32, tag="dtmp0", name="dtmp0")] * 2
        dnf = [pers.tile([128, 8], F32, tag=f"dnf{p}", name=f"dnf{p}") for p in range(2)]
        dnr = [pers.tile([128, 8], F32, tag=f"dnr{p}", name=f"dnr{p}") for p in range(2)]
        dnh = [pers.tile([128, 8], F16, tag=f"dnh{p}", name=f"dnh{p}") for p in range(2)]
        rrj = [pers.tile([1, N_TOK // 2], F16, tag=f"rr{p}", name=f"rr{p}") for p in range(2)]
        rm = pers.tile([128, N_TOK // 2], F16, tag="rm", name="rmsb")

        nc.sync.dma_start(ident[:], id_d[:])
        nc.sync.dma_start(qkb[:], qkb_d[:])
        nc.sync.dma_start(on16[:], on16_d[:])
        for h in range(NH):
            nc.sync.dma_start(k2[h][DH : DH + 1, :], on32_d[:])
        for m in range(16):
            nc.vector.memset(vsb[m][:, :, DH : DH + 1], 1.0)
        for t in range(2):
            nc.sync.dma_start(wo_sb[t][:], wo_d[t * 128 : (t + 1) * 128, :])

        # ---------------- pass-1 helper: per-query max for head h ---------
        def negm_half(h, half):
            # fold maxes for qt block [8*half, 8*half+8) into q2 row 64
            qs = slice(8 * half, 8 * half + 8)
            nc.vector.reduce_max(mcf[h][:, qs], mc2[h][:, qs, :], axis=XAX)
            pst = ps.tile([128, N_TOK // 2], F32, tag="s", name="pst")
            nc.tensor.transpose(pst[0:8, 0:128], mcf[h][:, qs], ident[:])
            nc.vector.tensor_scalar_mul(negm[h][half][:], pst[0:8, 0:128], -1.0)
            nc.sync.dma_start(
                q2[h][DH : DH + 1, half * 1024 : (half + 1) * 1024], negm[h][half][:]
            )

        def p1_qt(h, qt):
            for half in range(2):
                p = ps.tile([128, N_TOK // 2], F32, tag="s", name="p1")
                for kc in range(2):
                    nc.tensor.matmul(
                        p[:, kc * 512 : (kc + 1) * 512],
                        q2[h][0:DH, qt * 128 : (qt + 1) * 128],
                        k2[h][0:DH, half * 1024 + kc * 512 : half * 1024 + (kc + 1) * 512],
                        start=True,
                        stop=True,
                    )
                nc.vector.reduce_max(mc2[h][:, qt, half : half + 1], p[:], axis=XAX)
            if qt == 8:
                negm_half(h, 0)
            if qt == 15:
                negm_half(h, 1)

        # ---------------- AV + normalize helpers --------------------------
        def av_j_mm(h, j, m2, po):
            for u in range(2):  # fp16 moving operand caps at 512 cols
                nc.tensor.matmul(
                    po[:, u * 512 : (u + 1) * 512],
                    vsb[m2][:, h, :],
                    PT[:, _rs(h, m2), j * 1024 + u * 512 : j * 1024 + (u + 1) * 512],
                    start=(m2 == 0),
                    stop=(m2 == 15),
                )

        def o2q(h, j):
            return o2[h // 2][(h % 2) * DH : (h % 2) * DH + DH, j * 1024 : (j + 1) * 1024]

        def stash_j(h, j, po):
            # raw (unnormalized) AV output into o2; denominator row stashed
            # (via a partition-0 ping tile: DVE output bases must be aligned).
            # Normalization happens in one batched pass at the end.
            idx = 2 * h + j
            p = idx % 2
            nc.vector.tensor_copy(o2q(h, j), po[0:DH, :])
            nc.scalar.copy(dtmp[p][:], po[DH : DH + 1, :])
            nc.sync.dma_start(denall[idx : idx + 1, :], dtmp[p][:])

        def norm_j(h, j):
            p = j % 2
            idx = 2 * h + j
            nc.sync.dma_start(dnf[p][:], denall[idx : idx + 1, :])
            nc.vector.reciprocal(dnr[p][:], dnf[p][:])
            nc.vector.tensor_copy(dnh[p][:], dnr[p][:])
            nc.sync.dma_start(rrj[p][:], dnh[p][:])
            pr = ps.tile([128, N_TOK // 2], F32, tag="s", name="pr")
            for u in range(2):
                nc.tensor.matmul(
                    pr[0:DH, u * 512 : (u + 1) * 512],
                    on16[:], rrj[p][:, u * 512 : (u + 1) * 512],
                    start=True, stop=True,
                )
            rows = slice((h % 2) * DH, (h % 2) * DH + DH)
            nc.scalar.copy(rm[rows, :], pr[0:DH, :])
            nc.vector.tensor_mul(o2q(h, j), o2q(h, j), rm[rows, :])

        # ---------------- phase A: QKV projection (fp16) -------------------
        with (
            tc.tile_pool(name="xt", bufs=1) as xt_pool,
            tc.tile_pool(name="wgp", bufs=1) as wg_pool,
        ):
            xt_sb = xt_pool.tile([128, 8, N_TOK], F16, tag="xta", name="xta")
            xtr = xt_pool.tile([1, N_TOK], F16, tag="xtr", name="xtr")
            wg_sb = wg_pool.tile([128, 8, 3 * NH * DH], F16, tag="wga", name="wga")
            wgr = wg_pool.tile([1, 3 * NH * DH], F16, tag="wgr", name="wgr")
            for c in range(8):
                nc.sync.dma_start(wg_sb[:, c, :], wg_d[c * 128 : (c + 1) * 128, :])
                nc.sync.dma_start(xt_sb[:, c, :], xt_d[c * 128 : (c + 1) * 128, :])
            nc.sync.dma_start(xtr[:], xt_d[DIM : DIM + 1, :])
            nc.sync.dma_start(wgr[:], wg_d[DIM : DIM + 1, :])

            # q/k waves: ft 0/1 = q heads01/23 (cols 0:256), 2/3 = k (256:512)
            def qk_wave(ft):
                dst = q2 if ft < 2 else k2
                hb = 2 * (ft % 2)
                pw = [ps.tile([128, N_TOK // 2], F32, tag="s", name="pw") for _ in range(2)]
                for c in range(8):
                    for half in range(2):
                        for tj in range(2):
                            nc.tensor.matmul(
                                pw[half][:, tj * 512 : (tj + 1) * 512],
                                wg_sb[:, c, ft * 128 : (ft + 1) * 128],
                                xt_sb[:, c, half * 1024 + tj * 512 : half * 1024 + (tj + 1) * 512],
                                start=(c == 0),
                                stop=(c == 7),
                            )
                for half in range(2):
                    cols = slice(half * 1024, (half + 1) * 1024)
                    nc.scalar.activation(
                        dst[hb][0:DH, cols], pw[half][0:DH, :], IDENT,
                        bias=qkb[0:DH, ft : ft + 1],
                    )
                    nc.scalar.activation(
                        dst[hb + 1][0:DH, cols], pw[half][DH:128, :], IDENT,
                        bias=qkb[DH:128, ft : ft + 1],
                    )

            def v_tile(i):
                # two token-tiles (2i, 2i+1) of v into one psum tile
                pv = ps.tile([128, N_TOK // 2], F32, tag="s", name="pv")
                for c in range(9):
                    for u in range(2):
                        tt = 2 * i + u
                        lhs = (
                            xt_sb[:, c, tt * 128 : (tt + 1) * 128]
                            if c < 8
                            else xtr[:, tt * 128 : (tt + 1) * 128]
                        )
                        rhs = (
                            wg_sb[:, c, 2 * NH * DH : 3 * NH * DH]
                            if c < 8
                            else wgr[:, 2 * NH * DH : 3 * NH * DH]
                        )
                        nc.tensor.matmul(
                            pv[:, u * 512 : u * 512 + NH * DH],
                            lhs, rhs,
                            start=(c == 0),
                            stop=(c == 8),
                        )
                for u in range(2):
                    tt = 2 * i + u
                    nc.scalar.copy(
                        vsb[tt][:, :, 0:DH],
                        pv[:, u * 512 : u * 512 + NH * DH].rearrange(
                            "p (h d) -> p h d", h=NH
                        ),
                    )

            qk_wave(0)          # q heads 0,1
            qk_wave(2)          # k heads 0,1
            # interleave q23/k23 waves and v tiles with pass-1 of head 0
            qk_wave(1)
            p1_qt(0, 0)
            qk_wave(3)
            p1_qt(0, 1)
            for i in range(8):
                v_tile(i)
                if i < 7:
                    p1_qt(0, 2 + 2 * i)
                    p1_qt(0, 3 + 2 * i)

        # ---------------- attention: S^T + exp, pipelined ------------------
        # PT ring lives in its own pool so it reuses the SBUF freed by the
        # xt/wg pools above.
        ptp = ctx.enter_context(tc.tile_pool(name="ptp", bufs=1))
        PT = ptp.tile([128, RING, N_TOK], F16, tag="PT", name="PTsb")

        def pp(h):
            hp = h + 1 if h < 3 else None     # pass-1 head piggybacked
            ha = h - 1 if h >= 1 else None    # AV head piggybacked
            po = None
            for m in range(16):
                # AV for h-1 first: exp(h, m) reuses the PT ring slot that
                # AV(h-1) reads at this step, so its readers must be
                # emitted ahead of the overwrite.
                if ha is not None:
                    j = m // 8
                    if m % 8 == 0:
                        if m == 8:
                            stash_j(ha, 0, po)
                        po = pop.tile([DH + 1, N_TOK // 2], F32, tag="po", name="po")
                    av_j_mm(ha, j, 2 * (m % 8), po)
                    av_j_mm(ha, j, 2 * (m % 8) + 1, po)
                for half in range(2):
                    st = ps.tile([128, N_TOK // 2], F32, tag="s", name="st")
                    for j2 in range(2):
                        nc.tensor.matmul(
                            st[:, j2 * 512 : (j2 + 1) * 512],
                            k2[h][:, m * 128 : (m + 1) * 128],
                            q2[h][:, half * 1024 + j2 * 512 : half * 1024 + (j2 + 1) * 512],
                            start=True,
                            stop=True,
                        )
                    nc.scalar.activation(
                        PT[:, _rs(h, m), half * 1024 : (half + 1) * 1024],
                        st[:], EXP, scale=SCALE,
                    )
                if hp is not None:
                    p1_qt(hp, m)
            if ha is not None:
                stash_j(ha, 1, po)

        for h in range(NH):
            pp(h)

        # tail: AV for head 3, then one batched normalization pass
        for j in range(2):
            po = pop.tile([DH + 1, N_TOK // 2], F32, tag="po", name="po")
            for m2 in range(16):
                av_j_mm(3, j, m2, po)
            stash_j(3, j, po)
        for h in range(NH):
            for j in range(2):
                norm_j(h, j)

        # ---------------- out projection -----------------------------------
        with tc.tile_pool(name="ob", bufs=2) as obp:
            for dc in range(8):
                for half in range(2):
                    pout = ps.tile([128, N_TOK // 2], F32, tag="s", name="pout")
                    for ht in range(2):
                        for u in range(2):
                            nc.tensor.matmul(
                                pout[:, u * 512 : (u + 1) * 512],
                                wo_sb[ht][:, dc * 128 : (dc + 1) * 128],
                                o2[ht][:, half * 1024 + u * 512 : half * 1024 + (u + 1) * 512],
                                start=(ht == 0),
                                stop=(ht == 1),
                            )
                    ob = obp.tile([128, N_TOK // 2], F16, tag="ob", name="ob")
                    if (2 * dc + half) % 2 == 0:
                        nc.scalar.copy(ob[:], pout[:])
                    else:
                        nc.vector.tensor_copy(ob[:], pout[:])
                    nc.sync.dma_start(
                        out_d[dc * 128 : (dc + 1) * 128, half * 1024 : (half + 1) * 1024],
                        ob[:],
                    )
    nc.finalize()
    return nc


def _get_nc():
    if "nc" not in _CACHE:
        _CACHE["nc"] = build_nc()
    return _CACHE["nc"]


def kernel(x, Wqkv, bqkv, Wout, bout):
    x = np.asarray(x, np.float32)
    Wqkv = np.asarray(Wqkv, np.float32)
    bqkv = np.asarray(bqkv, np.float32)
    Wout = np.asarray(Wout, np.float32)
    bout = np.asarray(bout, np.float32)
    B = x.shape[0]
    ident = np.eye(128, dtype=np.float32)
    ones_row16 = np.ones((1, N_TOK), np.float16)

    in_maps = []
    for c in range(8):
        b, g = c // 4, c % 4
        xt = np.concatenate(
            [np.ascontiguousarray(x[b].T).astype(np.float16), ones_row16], 0
        )
        cols, bias = [], []
        for blk in range(3):  # q, k, v column blocks of Wqkv
            s = blk * DIM + g * NH * DH
            cols.append(Wqkv[:, s : s + NH * DH])
            bias.append(bqkv[s : s + NH * DH])
        wg = np.concatenate(
            [np.concatenate(cols, 1), np.concatenate(bias)[None, :]], 0
        ).astype(np.float16)
        # per-partition bias for q/k copies: col ft = 128-feature block
        # (ft 0,1 = q heads01/23; ft 2,3 = k heads01/23)
        qb = bqkv[g * 256 : (g + 1) * 256]
        kb = bqkv[DIM + g * 256 : DIM + (g + 1) * 256]
        qkb = np.stack([qb[:128], qb[128:], kb[:128], kb[128:]], 1).astype(np.float32)
        wo = np.ascontiguousarray(Wout[g * NH * DH : (g + 1) * NH * DH, :]).astype(
            np.float16
        )
        in_maps.append(
            {
                "xt": np.ascontiguousarray(xt),
                "wg": np.ascontiguousarray(wg),
                "qkb": np.ascontiguousarray(qkb),
                "wout": wo,
                "ident": ident,
                "ones32": np.ones((1, N_TOK), np.float32),
                "ones16": np.ones((1, DH), np.float16),
            }
        )

    _CACHE["last_in_maps"] = in_maps
    res = run_bass_kernel_spmd(_get_nc(), in_maps, list(range(8))).results
    out = np.empty((B, N_TOK, DIM), np.float32)
    for b in range(B):
        acc = res[4 * b]["out"].astype(np.float32)
        for g in range(1, 4):
            acc = acc + res[4 * b + g]["out"].astype(np.float32)
        out[b] = acc.T + bout[None, :]
    return out


if __name__ == "__main__":
    rng = np.random.default_rng(0)
    x = rng.standard_normal((2, N_TOK, DIM)).astype(np.float32)
    Wqkv = (rng.standard_normal((DIM, 3 * DIM)) * DIM**-0.5).astype(np.float32)
    bqkv = (rng.standard_normal(3 * DIM) * 0.02).astype(np.float32)
    Wout = (rng.standard_normal((DIM, DIM)) * DIM**-0.5).astype(np.float32)
    bout = (rng.standard_normal(DIM) * 0.02).astype(np.float32)
    o = kernel(x=x, Wqkv=Wqkv, bqkv=bqkv, Wout=Wout, bout=bout)
    print("kernel ran, out shape", o.shape)


# revision 20
# speedup vs baseline: 1.6649x; 1.1036x over previous
"""Trn2 Bass kernel for nn_Attention_16793322128104.

Sharding: 8 cores = 2 batches x 4 head-groups (4 heads each).
Per core: fp16 QKV projection (768 Wqkv cols), 4 attention heads with
exact per-query max (pass-1 fp32r S + vector reduce_max), softmax shift
folded into the S^T matmul as a 65th contraction row, exp on the scalar
engine into an fp16 PT ring, AV with ones-column denominator, fp16
partial out-projection. Host sums the 4 head-group partials per batch.

Schedule: uniform [128,1024] PSUM stream tiles (3-slot rotation, 6
banks) + a 2-bank AV accumulator pool. Pass-1 max for head h+2 and
AV for head h-1 are software-pipelined into head h's S^T/exp phase so
tensor/vector/scalar engines all stay busy.
"""

import sys
from contextlib import ExitStack

import numpy as np

sys.path.insert(0, "/opt/trn_rl_repo")

import concourse.bass as bass
import concourse.bacc as bacc
import concourse.mybir as mybir
from concourse import tile
from concourse.bass_utils import run_bass_kernel_spmd

F32 = mybir.dt.float32
F32R = mybir.dt.float32r
F16 = mybir.dt.float16
IDENT = mybir.ActivationFunctionType.Identity
EXP = mybir.ActivationFunctionType.Exp
XAX = mybir.AxisListType.X

N_TOK = 2048
DIM = 1024
NH = 4                # heads per core
DH = 64               # head dim
SCALE = 8.0           # sqrt(DH); reference MULTIPLIES by sqrt(d_head)
RING = 24             # PT ring slots (16 per head, AV trails one head)

_CACHE = {}


def _rs(h, m):
    return (16 * h + m) % RING


def build_nc():
    nc = bacc.Bacc()
    xt_d = nc.declare_dram_parameter("xt", [DIM + 1, N_TOK], F16, isOutput=False)
    wg_d = nc.declare_dram_parameter("wg", [DIM + 1, 3 * NH * DH], F16, isOutput=False)
    qkb_d = nc.declare_dram_parameter("qkb", [128, 4], F32, isOutput=False)
    wo_d = nc.declare_dram_parameter("wout", [2 * 128, DIM], F16, isOutput=False)
    id_d = nc.declare_dram_parameter("ident", [128, 128], F32, isOutput=False)
    on16_d = nc.declare_dram_parameter("ones16", [1, DH], F16, isOutput=False)
    out_d = nc.declare_dram_parameter("out", [DIM, N_TOK], F16, isOutput=True)

    with ExitStack() as ctx:
        tc = ctx.enter_context(tile.TileContext(nc))
        pers = ctx.enter_context(tc.tile_pool(name="pers", bufs=1))
        ps = ctx.enter_context(
            tc.tile_pool(name="ps", bufs=3, space=bass.MemorySpace.PSUM)
        )
        pop = ctx.enter_context(
            tc.tile_pool(name="pop", bufs=1, space=bass.MemorySpace.PSUM)
        )

        q2 = [pers.tile([DH + 1, N_TOK], F16, tag=f"q2{h}", name=f"q2{h}") for h in range(NH)]
        k2 = [pers.tile([DH + 1, N_TOK], F16, tag=f"k2{h}", name=f"k2{h}") for h in range(NH)]
        vsb = [pers.tile([128, NH, DH + 1], F16, tag=f"v{m}", name=f"v{m}") for m in range(16)]
        o2 = [pers.tile([128, N_TOK], F16, tag=f"o2{t}", name=f"o2{t}") for t in range(2)]
        wo_sb = [pers.tile([128, DIM], F16, tag=f"wo{t}", name=f"wo{t}") for t in range(2)]
        ident = pers.tile([128, 128], F32, tag="id", name="identsb")
        qkb = pers.tile([128, 4], F32, tag="qkb", name="qkbsb")
        on16 = pers.tile([1, DH], F16, tag="on16", name="on16sb")
        mc2 = [pers.tile([128, 16, 2], F32, tag=f"mc{h}", name=f"mc{h}") for h in range(NH)]
        mcf = [pers.tile([128, 16], F32, tag=f"mcf{h}", name=f"mcf{h}") for h in range(NH)]
        negm = [
            [pers.tile([8, 128], F16, tag=f"nm{h}{u}", name=f"nm{h}{u}") for u in range(2)]
            for h in range(NH)
        ]
        # deferred-normalization temporaries
        denall = pers.tile([8, N_TOK // 2], F32, tag="denall", name="denall")
        dtmp = [pers.tile([1, N_TOK // 2], F32, tag=f"dtmp{p}", name=f"dtmp{p}") for p in range(2)]
        dnf = [pers.tile([128, 8], F32, tag=f"dnf{p}", name=f"dnf{p}") for p in range(2)]
        dnr = [pers.tile([128, 8], F32, tag=f"dnr{p}", name=f"dnr{p}") for p in range(2)]
        dnh = [pers.tile([128, 8], F16, tag=f"dnh{p}", name=f"dnh{p}") for p in range(2)]
        rrj = [pers.tile([1, N_TOK // 2], F16, tag=f"rr{p}", name=f"rr{p}") for p in range(2)]
        rm = pers.tile([128, N_TOK // 2], F16, tag="rm", name="rmsb")

        nc.sync.dma_start(ident[:], id_d[:])
        nc.sync.dma_start(qkb[:], qkb_d[:])
        nc.sync.dma_start(on16[:], on16_d[:])
        for h in range(NH):
            nc.sync.dma_start(k2[h][DH : DH + 1, :], xt_d[DIM : DIM + 1, :])
        for m in range(16):
            nc.vector.memset(vsb[m][:, :, DH : DH + 1], 1.0)
        for t in range(2):
            nc.sync.dma_start(wo_sb[t][:], wo_d[t * 128 : (t + 1) * 128, :])

        # ---------------- pass-1 helper: per-query max for head h ---------
        def negm_half(h, half):
            # fold maxes for qt block [8*half, 8*half+8) into q2 row 64
            qs = slice(8 * half, 8 * half + 8)
            nc.vector.reduce_max(mcf[h][:, qs], mc2[h][:, qs, :], axis=XAX)
            pst = ps.tile([128, N_TOK // 2], F32, tag="s", name="pst")
            nc.tensor.transpose(pst[0:8, 0:128], mcf[h][:, qs], ident[:])
            nc.vector.tensor_scalar_mul(negm[h][half][:], pst[0:8, 0:128], -1.0)
            nc.sync.dma_start(
                q2[h][DH : DH + 1, half * 1024 : (half + 1) * 1024], negm[h][half][:]
            )

        def p1_qt(h, qt):
            for half in range(2):
                p = ps.tile([128, N_TOK // 2], F32, tag="s", name="p1")
                for kc in range(2):
                    nc.tensor.matmul(
                        p[:, kc * 512 : (kc + 1) * 512],
                        q2[h][0:DH, qt * 128 : (qt + 1) * 128],
                        k2[h][0:DH, half * 1024 + kc * 512 : half * 1024 + (kc + 1) * 512],
                        start=True,
                        stop=True,
                    )
                nc.vector.reduce_max(mc2[h][:, qt, half : half + 1], p[:], axis=XAX)
            if qt == 8:
                negm_half(h, 0)
            if qt == 15:
                negm_half(h, 1)

        # ---------------- AV + normalize helpers --------------------------
        def av_j_mm(h, j, m2, po):
            for u in range(2):  # fp16 moving operand caps at 512 cols
                nc.tensor.matmul(
                    po[:, u * 512 : (u + 1) * 512],
                    vsb[m2][:, h, :],
                    PT[:, _rs(h, m2), j * 1024 + u * 512 : j * 1024 + (u + 1) * 512],
                    start=(m2 == 0),
                    stop=(m2 == 15),
                )

        def o2q(h, j):
            return o2[h // 2][(h % 2) * DH : (h % 2) * DH + DH, j * 1024 : (j + 1) * 1024]

        def stash_j(h, j, po):
            # raw (unnormalized) AV output into o2; denominator row stashed
            # (via a partition-0 ping tile: DVE output bases must be aligned).
            # Normalization happens in one batched pass at the end.
            idx = 2 * h + j
            p = idx % 2
            nc.vector.tensor_copy(o2q(h, j), po[0:DH, :])
            nc.scalar.copy(dtmp[p][:], po[DH : DH + 1, :])
            nc.sync.dma_start(denall[idx : idx + 1, :], dtmp[p][:])

        def norm_j(h, j):
            p = j % 2
            idx = 2 * h + j
            nc.sync.dma_start(dnf[p][:], denall[idx : idx + 1, :])
            nc.vector.reciprocal(dnr[p][:], dnf[p][:])
            nc.vector.tensor_copy(dnh[p][:], dnr[p][:])
            nc.sync.dma_start(rrj[p][:], dnh[p][:])
            pr = ps.tile([128, N_TOK // 2], F32, tag="s", name="pr")
            for u in range(2):
                nc.tensor.matmul(
                    pr[0:DH, u * 512 : (u + 1) * 512],
                    on16[:], rrj[p][:, u * 512 : (u + 1) * 512],
                    start=True, stop=True,
                )
            rows = slice((h % 2) * DH, (h % 2) * DH + DH)
            nc.scalar.copy(rm[rows, :], pr[0:DH, :])
            nc.vector.tensor_mul(o2q(h, j), o2q(h, j), rm[rows, :])

        # ---------------- phase A: QKV projection (fp16) -------------------
        with (
            tc.tile_pool(name="xt", bufs=1) as xt_pool,
            tc.tile_pool(name="wgp", bufs=1) as wg_pool,
        ):
            xt_sb = xt_pool.tile([128, 8, N_TOK], F16, tag="xta", name="xta")
            xtr = xt_pool.tile([1, N_TOK], F16, tag="xtr", name="xtr")
            wg_sb = wg_pool.tile([128, 8, 3 * NH * DH], F16, tag="wga", name="wga")
            wgr = wg_pool.tile([1, 3 * NH * DH], F16, tag="wgr", name="wgr")
            for c in range(8):
                nc.sync.dma_start(wg_sb[:, c, :], wg_d[c * 128 : (c + 1) * 128, :])
                nc.sync.dma_start(xt_sb[:, c, :], xt_d[c * 128 : (c + 1) * 128, :])
            nc.sync.dma_start(xtr[:], xt_d[DIM : DIM + 1, :])
            nc.sync.dma_start(wgr[:], wg_d[DIM : DIM + 1, :])

            # q/k waves: ft 0/1 = q heads01/23 (cols 0:256), 2/3 = k (256:512)
            def qk_wave(ft):
                dst = q2 if ft < 2 else k2
                hb = 2 * (ft % 2)
                pw = [ps.tile([128, N_TOK // 2], F32, tag="s", name="pw") for _ in range(2)]
                for c in range(8):
                    for half in range(2):
                        for tj in range(2):
                            nc.tensor.matmul(
                                pw[half][:, tj * 512 : (tj + 1) * 512],
                                wg_sb[:, c, ft * 128 : (ft + 1) * 128],
                                xt_sb[:, c, half * 1024 + tj * 512 : half * 1024 + (tj + 1) * 512],
                                start=(c == 0),
                                stop=(c == 7),
                            )
                for half in range(2):
                    cols = slice(half * 1024, (half + 1) * 1024)
                    nc.scalar.activation(
                        dst[hb][0:DH, cols], pw[half][0:DH, :], IDENT,
                        bias=qkb[0:DH, ft : ft + 1],
                    )
                    nc.scalar.activation(
                        dst[hb + 1][0:DH, cols], pw[half][DH:128, :], IDENT,
                        bias=qkb[DH:128, ft : ft + 1],
                    )

            def v_tile(i):
                # two token-tiles (2i, 2i+1) of v into one psum tile
                pv = ps.tile([128, N_TOK // 2], F32, tag="s", name="pv")
                for c in range(9):
                    for u in range(2):
                        tt = 2 * i + u
                        lhs = (
                            xt_sb[:, c, tt * 128 : (tt + 1) * 128]
                            if c < 8
                            else xtr[:, tt * 128 : (tt + 1) * 128]
                        )
                        rhs = (
                            wg_sb[:, c, 2 * NH * DH : 3 * NH * DH]
                            if c < 8
                            else wgr[:, 2 * NH * DH : 3 * NH * DH]
                        )
                        nc.tensor.matmul(
                            pv[:, u * 512 : u * 512 + NH * DH],
                            lhs, rhs,
                            start=(c == 0),
                            stop=(c == 8),
                        )
                for u in range(2):
                    tt = 2 * i + u
                    nc.scalar.copy(
                        vsb[tt][:, :, 0:DH],
                        pv[:, u * 512 : u * 512 + NH * DH].rearrange(
                            "p (h d) -> p h d", h=NH
                        ),
                    )

            qk_wave(0)          # q heads 0,1
            qk_wave(2)          # k heads 0,1
            # interleave q23/k23 waves and v tiles with pass-1 of head 0
            qk_wave(1)
            p1_qt(0, 0)
            qk_wave(3)
            p1_qt(0, 1)
            for i in range(8):
                v_tile(i)
                if i < 7:
                    p1_qt(0, 2 + 2 * i)
                    p1_qt(0, 3 + 2 * i)

        # ---------------- attention: S^T + exp, pipelined ------------------
        # PT ring lives in its own pool so it reuses the SBUF freed by the
        # xt/wg pools above.
        ptp = ctx.enter_context(tc.tile_pool(name="ptp", bufs=1))
        PT = ptp.tile([128, RING, N_TOK], F16, tag="PT", name="PTsb")

        def pp(h):
            hp = h + 1 if h < 3 else None     # pass-1 head piggybacked
            ha = h - 1 if h >= 1 else None    # AV head piggybacked
            po = None
            for m in range(16):
                # AV for h-1 first: exp(h, m) reuses the PT ring slot that
                # AV(h-1) reads at this step, so its readers must be
                # emitted ahead of the overwrite.
                if ha is not None:
                    j = m // 8
                    if m % 8 == 0:
                        if m == 8:
                            stash_j(ha, 0, po)
                        po = pop.tile([DH + 1, N_TOK // 2], F32, tag="po", name="po")
                    av_j_mm(ha, j, 2 * (m % 8), po)
                    av_j_mm(ha, j, 2 * (m % 8) + 1, po)
                for half in range(2):
                    st = ps.tile([128, N_TOK // 2], F32, tag="s", name="st")
                    for j2 in range(2):
                        nc.tensor.matmul(
                            st[:, j2 * 512 : (j2 + 1) * 512],
                            k2[h][:, m * 128 : (m + 1) * 128],
                            q2[h][:, half * 1024 + j2 * 512 : half * 1024 + (j2 + 1) * 512],
                            start=True,
                            stop=True,
                        )
                    nc.scalar.activation(
                        PT[:, _rs(h, m), half * 1024 : (half + 1) * 1024],
                        st[:], EXP, scale=SCALE,
                    )
                if hp is not None:
                    p1_qt(hp, m)
            if ha is not None:
                stash_j(ha, 1, po)

        for h in range(NH):
            pp(h)

        # tail: AV for head 3 with earlier heads' norms interleaved
        norms = [(h, j) for h in range(3) for j in range(2)]
        ni = 0
        for j in range(2):
            po = pop.tile([DH + 1, N_TOK // 2], F32, tag="po", name="po")
            for m2 in range(16):
                av_j_mm(3, j, m2, po)
                if m2 % 5 == 4 and ni < len(norms):
                    norm_j(*norms[ni])
                    ni += 1
            stash_j(3, j, po)
        while ni < len(norms):
            norm_j(*norms[ni])
            ni += 1
        norm_j(3, 0)
        norm_j(3, 1)

        # ---------------- out projection -----------------------------------
        with tc.tile_pool(name="ob", bufs=3) as obp:
            for dc in range(8):
                for half in range(2):
                    pout = ps.tile([128, N_TOK // 2], F32, tag="s", name="pout")
                    for ht in range(2):
                        for u in range(2):
                            nc.tensor.matmul(
                                pout[:, u * 512 : (u + 1) * 512],
                                wo_sb[ht][:, dc * 128 : (dc + 1) * 128],
                                o2[ht][:, half * 1024 + u * 512 : half * 1024 + (u + 1) * 512],
                                start=(ht == 0),
                                stop=(ht == 1),
                            )
                    ob = obp.tile([128, N_TOK // 2], F16, tag="ob", name="ob")
                    if (2 * dc + half) % 2 == 0:
                        nc.scalar.copy(ob[:], pout[:])
                    else:
                        nc.vector.tensor_copy(ob[:], pout[:])
                    nc.sync.dma_start(
                        out_d[dc * 128 : (dc + 1) * 128, half * 1024 : (half + 1) * 1024],
                        ob[:],
                    )
    nc.finalize()
    return nc


def _get_nc():
    if "nc" not in _CACHE:
        _CACHE["nc"] = build_nc()
    return _CACHE["nc"]


def kernel(x, Wqkv, bqkv, Wout, bout):
    x = np.asarray(x, np.float32)
    Wqkv = np.asarray(Wqkv, np.float32)
    bqkv = np.asarray(bqkv, np.float32)
    Wout = np.asarray(Wout, np.float32)
    bout = np.asarray(bout, np.float32)
    B = x.shape[0]
    ident = np.eye(128, dtype=np.float32)
    ones_row16 = np.ones((1, N_TOK), np.float16)

    in_maps = []
    for c in range(8):
        b, g = c // 4, c % 4
        xt = np.concatenate(
            [np.ascontiguousarray(x[b].T).astype(np.float16), ones_row16], 0
        )
        cols, bias = [], []
        for blk in range(3):  # q, k, v column blocks of Wqkv
            s = blk * DIM + g * NH * DH
            cols.append(Wqkv[:, s : s + NH * DH])
            bias.append(bqkv[s : s + NH * DH])
        wg = np.concatenate(
            [np.concatenate(cols, 1), np.concatenate(bias)[None, :]], 0
        ).astype(np.float16)
        # per-partition bias for q/k copies: col ft = 128-feature block
        # (ft 0,1 = q heads01/23; ft 2,3 = k heads01/23)
        qb = bqkv[g * 256 : (g + 1) * 256]
        kb = bqkv[DIM + g * 256 : DIM + (g + 1) * 256]
        qkb = np.stack([qb[:128], qb[128:], kb[:128], kb[128:]], 1).astype(np.float32)
        wo = np.ascontiguousarray(Wout[g * NH * DH : (g + 1) * NH * DH, :]).astype(
            np.float16
        )
        in_maps.append(
            {
                "xt": np.ascontiguousarray(xt),
                "wg": np.ascontiguousarray(wg),
                "qkb": np.ascontiguousarray(qkb),
                "wout": wo,
                "ident": ident,
                "ones16": np.ones((1, DH), np.float16),
            }
        )

    _CACHE["last_in_maps"] = in_maps
    res = run_bass_kernel_spmd(_get_nc(), in_maps, list(range(8))).results
    out = np.empty((B, N_TOK, DIM), np.float32)
    for b in range(B):
        acc = res[4 * b]["out"].astype(np.float32)
        for g in range(1, 4):
            acc = acc + res[4 * b + g]["out"].astype(np.float32)
        out[b] = acc.T + bout[None, :]
    return out


if __name__ == "__main__":
    rng = np.random.default_rng(0)
    x = rng.standard_normal((2, N_TOK, DIM)).astype(np.float32)
    Wqkv = (rng.standard_normal((DIM, 3 * DIM)) * DIM**-0.5).astype(np.float32)
    bqkv = (rng.standard_normal(3 * DIM) * 0.02).astype(np.float32)
    Wout = (rng.standard_normal((DIM, DIM)) * DIM**-0.5).astype(np.float32)
    bout = (rng.standard_normal(DIM) * 0.02).astype(np.float32)
    o = kernel(x=x, Wqkv=Wqkv, bqkv=bqkv, Wout=Wout, bout=bout)
    print("kernel ran, out shape", o.shape)
